# revision 29
# baseline (speedup 1.0000x reference)
"""Multi-head causal attention (B=2, S=2048, D=1024, H=16, dh=64) on 8
Trainium2 NeuronCores.

Sharding: core i handles batch b = i//4 and head group g = i%4 (4 heads
each).  Per core everything is computed in a transposed layout:

  QT = Wq_g^T @ x_b^T          [256(hk), 2048(S)]   (bf16)
  KT = Wk_g^T @ x_b^T          [256(hk), 2048(S)]   (bf16)
  V  = x_b @ Wv_g              [2048(S), 4, 65]     (bf16; col 64 = ones)
  per chunk c (512 queries), head-pair hp, key block j (128 keys):
     scT[par] = KT_h[:,j]^T(lhsT) x QT_h[:,c]   -> PSUM [128, 2, 512]
     expT     = exp(scT/8) (* causal mask when j >= 4c)        (bf16)
     zT_h    += V_aug[j]^T(lhsT) x expT[par]    -> PSUM [65, 512]
                (row 64 accumulates the softmax denominator s)
     ztn      = zT[0:64] * broadcast(1/s)       [256(hk), 2048] (bf16)
  outT = Wo_g^T(lhsT) x ztn                     [1024(d), 2048] (bf16)

Emission is interleaved per chunk (proj0, attn0, proj1, attn1, proj2,
attn2, out0, proj3, attn3, out1..3) with phase-private PSUM tags, so the
Tile list-scheduler backfills projection / output-projection matmuls into
the PE gaps left while the Scalar engine works through the softmax EXPs.

Host: shards/transposes inputs, sums the 4 head-group partial outputs per
batch, adds b_O and the exact b_V fold (softmax rows sum to 1):
  out += b_O + sum_h b_V[h] @ W_O[h].
b_Q/b_K are folded on-device only when nonzero (separate compiled variant);
the common zero-bias case skips the Scalar-engine bias pass entirely.
"""
import numpy as np
import ml_dtypes

import concourse.bacc as bacc
import concourse.mybir as mybir
import concourse.tile as tile
from concourse.bass_utils import run_bass_kernel_spmd

f32 = mybir.dt.float32
bf16 = mybir.dt.bfloat16
AF = mybir.ActivationFunctionType

B, S, D, H, DH = 2, 2048, 1024, 16, 64
NCORES = 8
HG = 4                # heads per core
HK = HG * DH          # 256
CH = 512              # query chunk
NCH = S // CH         # 4
KB = 128              # key block
DT = D // 128         # 8

_CACHE = {}


def _build_nc(with_bias):
    nc = bacc.Bacc(None, target_bir_lowering=False, debug=False,
                   num_devices=NCORES)

    xt_d = nc.dram_tensor("xt", [128, DT, S], bf16, kind="ExternalInput")
    wq_d = nc.dram_tensor("wq", [128, DT, HK], bf16, kind="ExternalInput")
    wk_d = nc.dram_tensor("wk", [128, DT, HK], bf16, kind="ExternalInput")
    wv_d = nc.dram_tensor("wv", [128, DT, HK], bf16, kind="ExternalInput")
    wo_d = nc.dram_tensor("wo", [128, 2, D], bf16, kind="ExternalInput")
    mask_d = nc.dram_tensor("mask", [128, 4, 2, CH], bf16,
                            kind="ExternalInput")
    if with_bias:
        bq_d = nc.dram_tensor("bq", [128, 2], f32, kind="ExternalInput")
        bk_d = nc.dram_tensor("bk", [128, 2], f32, kind="ExternalInput")
    out_d = nc.dram_tensor("outT", [D, S], bf16, kind="ExternalOutput")

    with tile.TileContext(nc) as tc:
        with (
            tc.tile_pool(name="const", bufs=1) as cp,
            tc.tile_pool(name="big", bufs=1) as bp,
            tc.tile_pool(name="work", bufs=3) as wp,
            tc.tile_pool(name="psum", bufs=2, space="PSUM") as pp,
        ):
            # ---- loads
            wq = cp.tile([128, DT, HK], bf16)
            wk = cp.tile([128, DT, HK], bf16)
            wv = cp.tile([128, DT, HK], bf16)
            wo = cp.tile([128, 2, D], bf16)
            mask = cp.tile([128, 4, 2, CH], bf16)
            xt = bp.tile([128, DT, S], bf16)
            if with_bias:
                bq = cp.tile([128, 2], f32)
                bk = cp.tile([128, 2], f32)

            # xt streams on the HWDGE ring (sync) in per-di halves (2KB
            # lines keep the descriptor count low); weights go through the
            # SWDGE ring (gpsimd) in first-use order, split so the
            # first-use slice has a small DMA dependency.
            for di in range(DT):
                nc.sync.dma_start(xt[:, di, 0:S // 2], xt_d[:, di, 0:S // 2])
            for di in range(DT):
                nc.sync.dma_start(xt[:, di, S // 2:], xt_d[:, di, S // 2:])
            nc.gpsimd.dma_start(wq[:, 0, :], wq_d[:, 0, :])
            nc.gpsimd.dma_start(wq[:, 1:, :], wq_d[:, 1:, :])
            nc.gpsimd.dma_start(wk[:, 0, :], wk_d[:, 0, :])
            nc.gpsimd.dma_start(wk[:, 1:, :], wk_d[:, 1:, :])
            if with_bias:
                nc.gpsimd.dma_start(bq, bq_d[:])
                nc.gpsimd.dma_start(bk, bk_d[:])
            nc.gpsimd.dma_start(wv, wv_d[:])
            nc.gpsimd.dma_start(mask[:, 0], mask_d[:, 0])
            nc.gpsimd.dma_start(mask[:, 1:], mask_d[:, 1:])
            nc.gpsimd.dma_start(wo, wo_d[:])

            qt = bp.tile([128, 2, S], bf16)
            kt = bp.tile([128, 2, S], bf16)
            # V padded to 128 columns (cols 65.. zero) so the zT matmul's
            # stationary is 128-wide -> fast weight load / ldw pipelining
            v = bp.tile([128, S // KB, HG, 128], bf16)
            ztn = bp.tile([128, 2, S], bf16)

            # ones column for the denominator trick; zero the pad (DVE is
            # idle during the initial DMA window)
            nc.vector.memset(v[:, :, :, DH:DH + 1], 1.0)
            nc.vector.memset(v[:, :, :, DH + 1:], 0.0)
            # fp32 ones row for the tail's PE-side reciprocal broadcast
            ones_f = cp.tile([1, DH], f32)
            nc.vector.memset(ones_f, 1.0)

            def proj0():
                """Chunk-0 projections with all 8 psum banks: Q/K/V chains
                interleaved per di so each arriving xt slice feeds 8
                matmuls back to back (dense early PE stream while the
                input DMA is still the pacer)."""
                psq0 = pp.tile([128, CH], f32, tag="proj", bufs=2,
                               name="p0_q0")
                psq1 = pp.tile([128, CH], f32, tag="proj", bufs=2,
                               name="p0_q1")
                psk0 = pp.tile([128, CH], f32, tag="zt0", bufs=1,
                               name="p0_k0")
                psk1 = pp.tile([128, CH], f32, tag="zt1", bufs=1,
                               name="p0_k1")
                # V chains bank-aligned via padding (si stride = one full
                # psum bank): two interleaved accumulation groups sharing a
                # bank corrupt the accumulation, so each si owns its bank.
                psva = pp.tile([128, 2, 2, HG, DH], f32, tag="sc", bufs=2,
                               name="p0_va")
                psvb = pp.tile([128, 2, 2, HG, DH], f32, tag="sc", bufs=2,
                               name="p0_vb")
                vps = (psva[:, 0, 0], psva[:, 1, 0],
                       psvb[:, 0, 0], psvb[:, 1, 0])
                for di in range(DT):
                    st, sp = di == 0, di == DT - 1
                    nc.tensor.matmul(psq0, wq[:, di, 0:128],
                                     xt[:, di, 0:CH], start=st, stop=sp)
                    nc.tensor.matmul(psq1, wq[:, di, 128:256],
                                     xt[:, di, 0:CH], start=st, stop=sp)
                    nc.tensor.matmul(psk0, wk[:, di, 0:128],
                                     xt[:, di, 0:CH], start=st, stop=sp)
                    nc.tensor.matmul(psk1, wk[:, di, 128:256],
                                     xt[:, di, 0:CH], start=st, stop=sp)
                    for si in range(4):
                        nc.tensor.matmul(vps[si],
                                         xt[:, di, si * KB:(si + 1) * KB],
                                         wv[:, di, :], start=st, stop=sp)
                for m, psq in ((0, psq0), (1, psq1)):
                    if with_bias:
                        nc.scalar.activation(qt[:, m, 0:CH], psq,
                                             AF.Identity, bias=bq[:, m:m + 1])
                    else:
                        nc.scalar.activation(qt[:, m, 0:CH], psq, AF.Copy)
                for m, psk in ((0, psk0), (1, psk1)):
                    if with_bias:
                        nc.scalar.activation(kt[:, m, 0:CH], psk,
                                             AF.Identity, bias=bk[:, m:m + 1])
                    else:
                        nc.vector.tensor_copy(kt[:, m, 0:CH], psk)
                for si in range(4):
                    nc.vector.tensor_copy(v[:, si, :, 0:DH], vps[si])

            def proj(c):
                cs = c * CH
                for m in range(2):
                    ps_q = pp.tile([128, CH], f32, tag="proj", bufs=2,
                                   name=f"ps_q_{c}_{m}")
                    for di in range(DT):
                        nc.tensor.matmul(
                            ps_q, wq[:, di, m * 128:(m + 1) * 128],
                            xt[:, di, cs:cs + CH],
                            start=(di == 0), stop=(di == DT - 1))
                    if with_bias:
                        nc.scalar.activation(qt[:, m, cs:cs + CH], ps_q,
                                             AF.Identity, bias=bq[:, m:m + 1])
                    else:
                        # DVE, not ACT: a Copy in the strict-FIFO Scalar
                        # queue would delay the exp stream of the attention
                        # chunk this projection overlaps with.
                        nc.vector.tensor_copy(qt[:, m, cs:cs + CH], ps_q)
                    ps_k = pp.tile([128, CH], f32, tag="proj", bufs=2,
                                   name=f"ps_k_{c}_{m}")
                    for di in range(DT):
                        nc.tensor.matmul(
                            ps_k, wk[:, di, m * 128:(m + 1) * 128],
                            xt[:, di, cs:cs + CH],
                            start=(di == 0), stop=(di == DT - 1))
                    if with_bias:
                        nc.scalar.activation(kt[:, m, cs:cs + CH], ps_k,
                                             AF.Identity, bias=bk[:, m:m + 1])
                    else:
                        nc.vector.tensor_copy(kt[:, m, cs:cs + CH], ps_k)
                for si in range(4 * c, 4 * c + 4):
                    ps_v = pp.tile([128, HG, DH], f32, tag="proj", bufs=2,
                                   name=f"ps_v_{si}")
                    for di in range(DT):
                        nc.tensor.matmul(
                            ps_v, xt[:, di, si * KB:(si + 1) * KB],
                            wv[:, di, :],
                            start=(di == 0), stop=(di == DT - 1))
                    nc.vector.tensor_copy(v[:, si, :, 0:DH], ps_v)

            def attn(c):
                cs = c * CH
                nblk = 4 * c + 4       # key blocks for this chunk
                for hp in range(2):    # head pair (2hp, 2hp+1); m = hp
                    m = hp
                    zt0 = pp.tile([128, CH], f32, tag="zt0", bufs=1,
                                  name=f"zt0_{c}_{hp}")
                    zt1 = pp.tile([128, CH], f32, tag="zt1", bufs=1,
                                  name=f"zt1_{c}_{hp}")
                    zts = (zt0, zt1)
                    for j in range(nblk):
                        # diagonal blocks (t>=0): queries below 128t are
                        # fully masked -> compute only [128t, CH); the
                        # partially-masked region is just [128t, 128t+128)
                        t = j - 4 * c
                        ql = 128 * t if t > 0 else 0
                        sc = pp.tile([128, 2, CH], f32, tag="sc", bufs=2)
                        for par in range(2):
                            o = par * 64
                            nc.tensor.matmul(
                                sc[:, par, ql:],
                                kt[o:o + 64, m, j * KB:(j + 1) * KB],
                                qt[o:o + 64, m, cs + ql:cs + CH],
                                start=True, stop=True)
                        ex = wp.tile([128, 2, CH], bf16, tag="ex", bufs=6)
                        nc.scalar.activation(ex[:, :, ql:], sc[:, :, ql:],
                                             AF.Exp, scale=0.125)
                        if t >= 0:
                            qm = ql + 128
                            nc.vector.tensor_mul(ex[:, :, ql:qm],
                                                 ex[:, :, ql:qm],
                                                 mask[:, t, :, ql:qm])
                        for par in range(2):
                            h = 2 * hp + par
                            nc.tensor.matmul(
                                zts[par][:, ql:], v[:, j, h, :],
                                ex[:, par, ql:],
                                start=(j == 0), stop=(j == nblk - 1))
                    # normalize: ztn[h] = zt[0:64] / zt[64], read straight
                    # from the PSUM accumulator (the next head pair's zt
                    # matmuls trail its exps by much more than this chain,
                    # so holding the bank a little longer is free).
                    for par in range(2):
                        h = 2 * hp + par
                        o = par * 64
                        zsrc = zts[par]
                        srow = wp.tile([1, CH], f32, tag="srow", bufs=3,
                                       name=f"srow_{c}_{h}")
                        nc.vector.tensor_copy(srow, zsrc[DH:DH + 1, :])
                        rec = wp.tile([1, CH], f32, tag="rec", bufs=3,
                                      name=f"rec_{c}_{h}")
                        nc.vector.reciprocal_approx_fast(rec, srow)
                        bc = wp.tile([64, CH], f32, tag="bc", bufs=3,
                                     name=f"bc_{c}_{h}")
                        nc.gpsimd.partition_broadcast(bc, rec)
                        nc.vector.tensor_mul(ztn[o:o + 64, m, cs:cs + CH],
                                             zsrc[0:DH, :], bc)
                        if c == NCH - 1 and hp == 1 and par == 0:
                            # Two throwaway matmuls keyed to this chain's
                            # intermediates land inside the final serial
                            # normalize window, so the PE activity monitor
                            # never re-throttles the clock and the last
                            # output projection runs at full rate.  Their
                            # results are unused; drains go to the (idle)
                            # Scalar engine so the DVE chain isn't delayed.
                            warm1 = pp.tile([DH, CH], f32, tag="proj",
                                            bufs=2, name="warm1")
                            nc.tensor.matmul(warm1, ones_f, rec,
                                             start=True, stop=True)
                            w1 = wp.tile([1, CH], f32, tag="rec", bufs=3,
                                         name="wdrain1")
                            nc.scalar.copy(w1, warm1[0:1, :])
                            warm2 = pp.tile([128, CH], f32, tag="proj",
                                            bufs=2, name="warm2")
                            nc.tensor.matmul(warm2, wo[0:64, 1, 0:128],
                                             ztn[0:64, 1, cs:cs + CH],
                                             start=True, stop=True)
                            w2 = wp.tile([1, CH], f32, tag="rec", bufs=3,
                                         name="wdrain2")
                            nc.scalar.copy(w2, warm2[0:1, :])

            def outproj(c):
                cs = c * CH
                for dt_i in range(DT):
                    ps_o = pp.tile([128, CH], f32, tag="proj", bufs=2,
                                   name=f"ps_o_{c}_{dt_i}")
                    for m in range(2):
                        nc.tensor.matmul(
                            ps_o, wo[:, m, dt_i * 128:(dt_i + 1) * 128],
                            ztn[:, m, cs:cs + CH],
                            start=(m == 0), stop=(m == 1))
                    ost = wp.tile([128, CH], bf16, tag="ost", bufs=4)
                    nc.vector.tensor_copy(ost, ps_o)
                    nc.sync.dma_start(
                        out_d[dt_i * 128:(dt_i + 1) * 128, cs:cs + CH], ost)

            # Interleaved emission: attn(c) gets priority over proj(c+1) /
            # outproj so the scheduler backfills projection matmuls into
            # EXP-paced attention gaps; outproj(c) is emitted late enough
            # that its psum-tag slots never gate an upcoming proj chunk.
            proj0()
            attn(0)
            proj(1)
            attn(1)
            proj(2)
            attn(2)
            outproj(0)
            proj(3)
            attn(3)
            outproj(1)
            outproj(2)
            outproj(3)

    nc.compile()
    return nc


def _tile128(a, inner_shape):
    """[N*128, ...] -> [128, N, ...] partition-major layout."""
    n = a.shape[0] // 128
    return np.ascontiguousarray(
        a.reshape((n, 128) + a.shape[1:]).swapaxes(0, 1)).reshape(
            (128, n) + inner_shape)


def _prep_core(x, W_Q, W_K, W_V, W_O, b_Q, b_K, b, g, with_bias):
    hs = slice(g * HG, (g + 1) * HG)
    bfl = ml_dtypes.bfloat16

    xtp = np.ascontiguousarray(x[b].T)                       # [D, S]
    xt = _tile128(xtp, (S,)).astype(bfl)                     # [128, DT, S]

    def prep_w(w):                                           # [H,D,dh] slice
        wc = np.ascontiguousarray(
            w[hs].transpose(1, 0, 2).reshape(D, HK))         # [D, HK]
        return _tile128(wc, (HK,)).astype(bfl)               # [128, DT, HK]

    wq, wk, wv = prep_w(W_Q), prep_w(W_K), prep_w(W_V)
    woc = W_O[hs].reshape(HK, D)                             # [HK, D]
    wo = _tile128(woc, (D,)).astype(bfl)                     # [128, 2, D]

    r = np.arange(128)[:, None, None]
    f = np.arange(CH)[None, None, :]
    t = np.arange(4)[None, :, None]
    m3 = (f >= r + 128 * t)                                  # [128, 4, CH]
    mask = np.repeat(m3[:, :, None, :], 2, axis=2).astype(bfl)

    out = {"xt": xt, "wq": wq, "wk": wk, "wv": wv, "wo": wo, "mask": mask}
    if with_bias:
        out["bq"] = np.ascontiguousarray(
            b_Q[hs].reshape(HK).reshape(2, 128).T).astype(np.float32)
        out["bk"] = np.ascontiguousarray(
            b_K[hs].reshape(HK).reshape(2, 128).T).astype(np.float32)
    return out


def kernel(x, W_Q, W_K, W_V, W_O, b_Q, b_K, b_V, b_O, **run_kwargs):
    x = np.asarray(x, dtype=np.float32)
    W_Q = np.asarray(W_Q, dtype=np.float32)
    W_K = np.asarray(W_K, dtype=np.float32)
    W_V = np.asarray(W_V, dtype=np.float32)
    W_O = np.asarray(W_O, dtype=np.float32)
    b_Q = np.asarray(b_Q, dtype=np.float32)
    b_K = np.asarray(b_K, dtype=np.float32)
    b_V = np.asarray(b_V, dtype=np.float32)
    b_O = np.asarray(b_O, dtype=np.float32)

    with_bias = bool(np.any(b_Q) or np.any(b_K))
    key = ("bias" if with_bias else "nobias")
    if key not in _CACHE:
        _CACHE[key] = _build_nc(with_bias)
    nc = _CACHE[key]

    in_maps = []
    for i in range(NCORES):
        b, g = i // HG, i % HG
        in_maps.append(
            _prep_core(x, W_Q, W_K, W_V, W_O, b_Q, b_K, b, g, with_bias))

    res = run_bass_kernel_spmd(nc, in_maps, core_ids=list(range(NCORES)),
                               **run_kwargs)

    # exact fold of b_V through W_O (softmax rows sum to 1), plus b_O
    bias = (b_O.astype(np.float64)
            + b_V.reshape(H * DH).astype(np.float64)
            @ W_O.reshape(H * DH, D).astype(np.float64)).astype(np.float32)

    out = np.zeros((B, S, D), dtype=np.float32)
    for i in range(NCORES):
        b = i // HG
        out[b] += res.results[i]["outT"].astype(np.float32).T
    out += bias[None, None, :]
    if run_kwargs:
        return out, res
    return out
